# revision 18
# baseline (speedup 1.0000x reference)
"""Trainium2 Bass kernel for nn_LocalExperts (MoE expert-parallel FFN).

Reference computation (per full input):
    x  [T=16384, D=1024] -> reshape [E=8, C=2048, D]
    h  = gelu(x @ w1[e] + b1[e])     w1 [E, D, F=4096]
    y  = h @ w2[e] + b2[e]           w2 [E, F, D]
    out[T, D]

Sharding: expert parallelism across 8 NeuronCores. Expert e's tokens are
exactly rows [e*C:(e+1)*C] of the input, so core e gets that token slice
plus w1[e], b1[e], w2[e], b2[e]. No collectives needed; outputs are
reassembled on the host.

All matmul operands are bf16 (fp32 PSUM accumulation): the PE streams
1 row/cycle at the warm 2.4 GHz clock, so the march of 2048 N=512
matmuls has a hard floor of ~443 us; fp8 double-pump cannot meet the
accuracy gate and TRN2 PSUM is fp32-only (N<=512).  The baseline ran
the march at the floor but wasted ~29 us before it and ~5 us after:

  - ~6.4 us framework preamble (fixed, counted in the graded window)
  - fp32 identity warmups ran in slow LOW_HIGH mode and blocked the PE
    queue; HAM un-throttled only at 11.5 us
  - 27 startup dma_starts at ~0.7 us trigger cost each serialized on
    the 2 HWDGE queues, starving both the PE (4.4 us stall at 27 us, a
    HAM re-throttle) and the weight prefetch
  - 6.7 us tail: final DVE add blocked behind queue bookkeeping, then a
    ~0.7 us writeback trigger + 128 KB transfer + end barrier

This version restructures startup and tail around those measurements:

  - x is pre-transposed per expert to xt [128, cc, di, 512] (c-major) so
    every DMA line is 8 KB and the whole xt ships as 5 dma_starts on the
    Sync queue, which carries nothing else until the final writebacks.
  - bf16 zero-tile warmup matmuls (N=512) warm the HAM clock during the
    preamble-to-first-data window without blocking real work.
  - fci=0 GEMM1 runs cc-major in two f-half phases (A: fti01, B: fti23)
    so compute starts after only ~768 KB (first xt chunk + w1 f-half)
    has landed and each later 1 MB xt chunk unlocks 16 matmuls.
  - All weight loads ride the Scalar queue, paced by the gelu drains
    emitted between them (phase A/B stage w1-half2, w2c0, chunk-1; each
    later fci's chunks sit behind the previous fci's gelus), so they
    never steal HBM bandwidth from the startup-critical xt stream.
  - b1/b2 are packed into one [128, 40] f32 tensor = one dma trigger.
  - Final fci GEMM2 drains per-cc into [128,1024] bf16 staging pairs;
    the very last c-block is split 384/128 so the post-last-matmul
    chain is one 128-col DVE add + one small DMA.

Per-core loop (C=2048 tokens, one expert), F chunked by FC=512:
  GEMM1: ht[f, c] = gelu(w1cT-tiles @ xT + b1)    (PSUM acc over 8 d-tiles,
                                                   4 psum banks = 4 c-chunks
                                                   share each stationary)
  GEMM2: yT[d, c] += w2c-tiles @ ht               (PSUM acc over 4 f-tiles,
                                                   DVE acc over chunks)
"""

import os
from contextlib import ExitStack

import ml_dtypes
import numpy as np

import concourse.bass as bass
import concourse.tile as tile
from concourse import bacc
from concourse import mybir
from concourse.bass import ds, ts
from concourse.bass_utils import run_bass_kernel_spmd

AFT = mybir.ActivationFunctionType

E = 8
D = 1024
F = 4096
T = 16384
C = T // E          # tokens per core
P = 128

D_T = D // P        # 8 d-tiles
FC = 512            # F chunk per iteration
FC_T = FC // P      # 4 f-tiles per chunk
N_FC = F // FC      # 8 chunks
NFREE = 512         # matmul moving free dim (one PSUM bank of fp32)
NCC = C // NFREE    # 4 c-chunks

# test-only: CoreSim lacks Gelu; "tanh" swaps the activation for sim gating
ACT_FN = os.environ.get("KERNEL_ACT", "gelu")


def _emit(ctx: ExitStack, tc: tile.TileContext, x, w1h, w1, w2, bt, y):
    nc = tc.nc
    f32 = mybir.dt.float32
    bf16 = mybir.dt.bfloat16

    consts = ctx.enter_context(tc.tile_pool(name="consts", bufs=1))
    xt_pool = ctx.enter_context(tc.tile_pool(name="xt", bufs=1))
    w1c0_pool = ctx.enter_context(tc.tile_pool(name="w1c0", bufs=1))
    yacc_pool = ctx.enter_context(tc.tile_pool(name="yacc", bufs=1))
    w1_pool = ctx.enter_context(tc.tile_pool(name="w1c", bufs=2))
    w2_pool = ctx.enter_context(tc.tile_pool(name="w2c", bufs=2))
    ht_pool = ctx.enter_context(tc.tile_pool(name="ht", bufs=2))
    ys_pool = ctx.enter_context(tc.tile_pool(name="ys", bufs=4))
    mm_psum = ctx.enter_context(tc.tile_pool(name="mmp", bufs=8, space="PSUM"))

    act_fn = AFT.Tanh if ACT_FN == "tanh" else AFT.Gelu_apprx_tanh

    # ---- HAM warmup: zero-tile bf16 matmuls keep the PE busy from right
    # after the preamble until the first real data lands (~3 us), so the
    # clock gate opens at ~10 us instead of 11.5 and real matmuls never
    # run at the cold 1.2 GHz rate.
    zq = consts.tile([P, NFREE], bf16)
    nc.vector.memset(zq[:], 0.0)
    bt_s = consts.tile([P, 40], f32)    # cols 0:32 = b1t, cols 32:40 = b2t
    scr = consts.tile([P, NFREE], bf16)  # DMA-prewarm scratch
    # 10 x N=512 cold spans ~4.3 us — reliably one full HAM SHORT window
    # (8 spanned ~3.4 us, a coin flip on the free-running window phase),
    # draining right as the first real payload lands (~12 us).
    warm_ps = mm_psum.tile([P, NFREE], f32, tag="mm", name="warm")
    for _ in range(10):
        nc.tensor.matmul(warm_ps[:], lhsT=zq[:, :P], rhs=zq[:],
                         start=True, stop=True)

    # ---- DMA prewarm: tiny HBM touches (16 descriptors each, no SBUF
    # dependency so they fire the moment the preamble ends) to take the
    # cold-path descriptor penalty before the real payload.
    nc.sync.dma_start(scr[0:16, 0:64], x[0:16, 0, 0, 0:64])
    nc.scalar.dma_start(scr[16:32, 0:64], x[16:32, 0, 1, 0:64])

    # ---- startup DMA.  A single HWDGE ring drains ~1 batch at a time at
    # only ~150-170 GB/s, so the startup-critical stream must be
    # interleaved across BOTH rings in need-order: cc0+cc1 xt halves and
    # the w1 f-halves land just ahead of the cc-major GEMM1 consumption
    # (~1 MB per 3.5 us once warm).  Everything not needed until later
    # (w2c0, chunk-1) is staged behind gelu drains below.
    xt = xt_pool.tile([P, NCC, D_T, NFREE], bf16, tag="xt")
    w1cA = w1c0_pool.tile([P, D_T, FC // 2], bf16, tag="w1cA")  # fti 0,1
    w1cB = w1c0_pool.tile([P, D_T, FC // 2], bf16, tag="w1cB")  # fti 2,3
    # ~256 KB pieces (1.7 us each per ring) alternate between the rings in
    # exact first-use order, so each piece lands just as the cc-major
    # GEMM1 consumption reaches it.
    ht0 = ht_pool.tile([P, FC_T, C], bf16, tag="ht", name="ht0")
    w2c0 = w2_pool.tile([P, FC_T, D], bf16, tag="w2c", name="w2c0")
    w1c1 = w1_pool.tile([P, D_T, FC], bf16, tag="w1c", name="w1c1")
    w2c1 = w2_pool.tile([P, FC_T, D], bf16, tag="w2c", name="w2c1")
    nc.sync.dma_start(w1cA[:, 0:4, :], w1h[0][:, 0:4, :])
    nc.scalar.dma_start(xt[:, 0, 0:2, :], x[:, 0, 0:2, :])
    nc.sync.dma_start(xt[:, 0, 2:4, :], x[:, 0, 2:4, :])
    nc.scalar.dma_start(w1cB[:, 0:4, :], w1h[1][:, 0:4, :])
    nc.sync.dma_start(w1cA[:, 4:8, :], w1h[0][:, 4:8, :])
    nc.scalar.dma_start(w1cB[:, 4:8, :], w1h[1][:, 4:8, :])
    nc.sync.dma_start(xt[:, 0, 4:6, :], x[:, 0, 4:6, :])
    nc.scalar.dma_start(xt[:, 0, 6:8, :], x[:, 0, 6:8, :])
    nc.sync.dma_start(xt[:, 1, 0:2, :], x[:, 1, 0:2, :])
    nc.scalar.dma_start(xt[:, 1, 2:4, :], x[:, 1, 2:4, :])
    nc.sync.dma_start(bt_s[:], bt[:])
    nc.sync.dma_start(xt[:, 1, 4:6, :], x[:, 1, 4:6, :])
    nc.scalar.dma_start(xt[:, 1, 6:8, :], x[:, 1, 6:8, :])
    # later-needed payload continues in ring-FIFO order behind the
    # critical stream — the per-ring drain order is the prefetch pacing
    nc.sync.dma_start(w2c0[:, 0:2, :], w2[0][:, 0:2, :])
    nc.scalar.dma_start(w2c0[:, 2:4, :], w2[0][:, 2:4, :])
    nc.sync.dma_start(xt[:, 2, 0:4, :], x[:, 2, 0:4, :])
    nc.scalar.dma_start(xt[:, 2, 4:8, :], x[:, 2, 4:8, :])
    nc.sync.dma_start(xt[:, 3, 0:4, :], x[:, 3, 0:4, :])
    nc.scalar.dma_start(xt[:, 3, 4:8, :], x[:, 3, 4:8, :])
    nc.sync.dma_start(w1c1[:], w1[1])
    nc.scalar.dma_start(w2c1[:], w2[1])

    yacc = yacc_pool.tile([P, D_T, C], f32, tag="yacc")

    # ---- fci=0, cc-major with all 4 f-tiles per cc so each arriving xt
    # piece opens a full runway, and with GEMM2 cc-blocks (which consume
    # no new HBM bytes once w2c0 is in) interleaved between GEMM1 blocks.
    # That drops the startup demand rate from ~295 GB/s (a knife-edge
    # against the two rings' ~300 GB/s) to ~230 GB/s, giving every piece
    # >=3.4 us of arrival slack.
    def g1_block(cc):
        pss = [mm_psum.tile([P, NFREE], f32, tag="mm", name=f"p{f}")
               for f in range(FC_T)]
        # The PE queue is FIFO, so emit each cc's matmuls in the order
        # their input pieces arrive, lest a later-piece-dependent matmul
        # head-of-line-block runnable ones.
        if cc == 0:
            order = [(di, f) for f in (0, 1) for di in (0, 1)]
            order += [(di, f) for f in (0, 1) for di in (2, 3)]
            order += [(di, f) for f in (2, 3) for di in range(4)]
            order += [(di, f) for f in range(FC_T) for di in (4, 5)]
            order += [(di, f) for f in range(FC_T) for di in (6, 7)]
        elif cc == 1:
            order = [(di, f) for dp in ((0, 1), (2, 3), (4, 5), (6, 7))
                     for f in range(FC_T) for di in dp]
        else:
            order = [(di, f) for dh in (range(4), range(4, D_T))
                     for f in range(FC_T) for di in dh]
        for di, f in order:
            w1ch = w1cA if f < 2 else w1cB
            nc.tensor.matmul(
                pss[f][:],
                lhsT=w1ch[:, di, ds((f % 2) * P, P)],
                rhs=xt[:, cc, di, :],
                start=(di == 0),
                stop=(di == D_T - 1),
            )
        for f in range(FC_T):
            nc.scalar.activation(
                ht0[:, f, ds(cc * NFREE, NFREE)],
                pss[f][:],
                act_fn,
                bias=bt_s[:, f : f + 1],
                scale=1.0,
            )

    def g2_block(cc):
        # GEMM2 for one c-chunk: 4 psum banks (d-tiles) accumulate over
        # fti; yacc initialized with b2 on DVE.
        for dh in (range(0, 4), range(4, D_T)):
            pss = [mm_psum.tile([P, NFREE], f32, tag="mm", name=f"q{j}")
                   for j in range(len(dh))]
            for fti in range(FC_T):
                for j, dti in enumerate(dh):
                    nc.tensor.matmul(
                        pss[j][:],
                        lhsT=w2c0[:, fti, ds(dti * P, P)],
                        rhs=ht0[:, fti, ds(cc * NFREE, NFREE)],
                        start=(fti == 0),
                        stop=(fti == FC_T - 1),
                    )
            for j, dti in enumerate(dh):
                nc.vector.tensor_scalar_add(
                    out=yacc[:, dti, ds(cc * NFREE, NFREE)],
                    in0=pss[j][:],
                    scalar1=bt_s[:, 32 + dti : 33 + dti],
                )

    g1_block(0)
    g1_block(1)
    g2_block(0)
    g1_block(2)
    g2_block(1)
    g1_block(3)
    g2_block(2)
    g2_block(3)

    # ---- fci = 1..7 ----
    for fci in range(1, N_FC):
        if fci == 1:
            w1c, w2c = w1c1, w2c1
        else:
            # Both chunk loads ride the Scalar queue: they sit behind the
            # previous fci's gelus, which lands them ~1.5 fci ahead of
            # use without competing with the startup xt stream.
            w1c = w1_pool.tile([P, D_T, FC], bf16, tag="w1c")
            nc.scalar.dma_start(w1c[:], w1[fci])
            w2c = w2_pool.tile([P, FC_T, D], bf16, tag="w2c")
            nc.scalar.dma_start(w2c[:], w2[fci])

        # ---- GEMM1: ht[f, c] = gelu(sum_d w1[d, f]^T x^T[d, c] + b1[f])
        ht = ht_pool.tile([P, FC_T, C], bf16, tag="ht")
        for fti in range(FC_T):
            pss = [mm_psum.tile([P, NFREE], f32, tag="mm", name=f"ps{g}")
                   for g in range(NCC)]
            for di in range(D_T):
                for cci in range(NCC):
                    nc.tensor.matmul(
                        pss[cci][:],
                        lhsT=w1c[:, di, ds(fti * P, P)],
                        rhs=xt[:, cci, di, :],
                        start=(di == 0),
                        stop=(di == D_T - 1),
                    )
            for cci in range(NCC):
                ft_g = fci * FC_T + fti
                nc.scalar.activation(
                    ht[:, fti, ds(cci * NFREE, NFREE)],
                    pss[cci][:],
                    act_fn,
                    bias=bt_s[:, ft_g : ft_g + 1],
                    scale=1.0,
                )

        # ---- GEMM2 (output-transposed): yT[d, c] += sum_f w2[f, d]^T h[f, c]
        last = fci == N_FC - 1
        if not last:
            for dti in range(D_T):
                pss = [mm_psum.tile([P, NFREE], f32, tag="mm", name=f"ps{cc}")
                       for cc in range(NCC)]
                for fti in range(FC_T):
                    for cci in range(NCC):
                        nc.tensor.matmul(
                            pss[cci][:],
                            lhsT=w2c[:, fti, ds(dti * P, P)],
                            rhs=ht[:, fti, ds(cci * NFREE, NFREE)],
                            start=(fti == 0),
                            stop=(fti == FC_T - 1),
                        )
                for cci in range(NCC):
                    ya = yacc[:, dti, ds(cci * NFREE, NFREE)]
                    nc.vector.tensor_add(out=ya, in0=ya, in1=pss[cci][:])
        else:
            # last chunk: singleton psum groups so every c-piece drains and
            # DMAs right after its own matmuls.  Pieces pair into 1 KB-per-
            # partition bf16 staging tiles (one trigger per 256 KB).  The
            # final c-block shrinks to 384 then 128 columns, and the 128-col
            # drain is split by partition (96/32) — after the last matmul
            # only two short DVE adds and an 8 KB DMA remain before the end
            # barrier's wait on the HBM write ack.
            for dti in range(D_T):
                if dti < D_T - 1:
                    pieces = [(cc * NFREE, NFREE) for cc in range(NCC)]
                else:
                    pieces = [(0, 512), (512, 512), (1024, 512),
                              (1536, 384), (1920, 128)]
                ys_cur = None
                ys_fill = 0
                for pi, (c0, cw) in enumerate(pieces):
                    final = dti == D_T - 1 and pi == len(pieces) - 1
                    ps = mm_psum.tile([P, NFREE], f32, tag="mm")
                    for fti in range(FC_T):
                        nc.tensor.matmul(
                            ps[:, :cw],
                            lhsT=w2c[:, fti, ds(dti * P, P)],
                            rhs=ht[:, fti, ds(c0, cw)],
                            start=(fti == 0),
                            stop=(fti == FC_T - 1),
                        )
                    if ys_cur is None:
                        ys_cur = ys_pool.tile([P, 1024], bf16, tag="ys")
                        ys_fill = 0
                    if final:
                        for p0, pn, q in ((0, 96, nc.sync),
                                          (96, 32, nc.scalar)):
                            nc.vector.tensor_add(
                                out=ys_cur[p0 : p0 + pn, ds(0, cw)],
                                in0=yacc[p0 : p0 + pn, dti, ds(c0, cw)],
                                in1=ps[p0 : p0 + pn, :cw],
                            )
                            q.dma_start(
                                y[dti][p0 : p0 + pn, ds(c0, cw)],
                                ys_cur[p0 : p0 + pn, ds(0, cw)],
                            )
                        continue
                    nc.vector.tensor_add(
                        out=ys_cur[:, ds(ys_fill, cw)],
                        in0=yacc[:, dti, ds(c0, cw)],
                        in1=ps[:, :cw],
                    )
                    ys_fill += cw
                    flush = ys_fill >= 1024 or pi == len(pieces) - 1 or (
                        dti == D_T - 1 and pi >= 2
                    )
                    if flush:
                        # alternate writeback queues; both are idle here so
                        # transfers overlap instead of serializing
                        q = nc.sync if (dti * 4 + pi) % 2 == 0 else nc.scalar
                        q.dma_start(
                            y[dti][:, ds(c0 + cw - ys_fill, ys_fill)],
                            ys_cur[:, :ys_fill],
                        )
                        ys_cur = None
                        ys_fill = 0


_NC_CACHE = None


def build_bass():
    global _NC_CACHE
    if _NC_CACHE is not None:
        return _NC_CACHE
    nc = bacc.Bacc("TRN2", target_bir_lowering=False, debug=False)
    f32 = mybir.dt.float32
    bf16 = mybir.dt.bfloat16
    x = nc.dram_tensor("x", [P, NCC, D_T, NFREE], bf16, kind="ExternalInput").ap()
    w1h = nc.dram_tensor("w1h", [2, P, D_T, FC // 2], bf16, kind="ExternalInput").ap()
    w1 = nc.dram_tensor("w1", [N_FC, P, D_T, FC], bf16, kind="ExternalInput").ap()
    w2 = nc.dram_tensor("w2", [N_FC, P, FC_T, D], bf16, kind="ExternalInput").ap()
    bt = nc.dram_tensor("bt", [P, 40], f32, kind="ExternalInput").ap()
    y = nc.dram_tensor("y", [D_T, P, C], bf16, kind="ExternalOutput").ap()
    with tile.TileContext(nc) as tc:
        with ExitStack() as ctx:
            _emit(ctx, tc, x, w1h, w1, w2, bt, y)
    nc.compile()
    _NC_CACHE = nc
    return nc


def _prep_core(x_e, w1_e, b1_e, w2_e, b2_e):
    bf16 = ml_dtypes.bfloat16
    # xt[p, cc, di, j] = x[cc*512 + j, di*128 + p]  (c-major, 8 KB lines)
    xt = np.ascontiguousarray(
        x_e.reshape(NCC, NFREE, D_T, P).transpose(3, 0, 2, 1).astype(bf16)
    )
    # first chunk split by f-column pair: w1h[h, p, di, j] = w1[di*128+p, h*256+j]
    w1hp = np.ascontiguousarray(
        w1_e[:, :FC].reshape(D_T, P, 2, FC // 2).transpose(2, 1, 0, 3).astype(bf16)
    )
    # w1 chunk layout [fci, p, di, fj]: element w1[di*128+p, fci*512+fj]
    w1p = np.ascontiguousarray(
        w1_e.reshape(D_T, P, N_FC, FC).transpose(2, 1, 0, 3).astype(bf16)
    )
    # w2 chunk layout [fci, p, fti, d]: element w2[(fci*4+fti)*128+p, d]
    w2p = np.ascontiguousarray(
        w2_e.reshape(N_FC, FC_T, P, D).transpose(0, 2, 1, 3).astype(bf16)
    )
    # packed biases: bt[p, 0:32] = b1[ft*128+p], bt[p, 32+dt] = b2[dt*128+p]
    bt = np.concatenate(
        [b1_e.reshape(F // P, P).T, b2_e.reshape(D // P, P).T], axis=1
    ).astype(np.float32)
    return {"x": xt, "w1h": w1hp, "w1": w1p, "w2": w2p, "bt": np.ascontiguousarray(bt)}


def _in_maps(inputs, w1, b1, w2, b2):
    return [
        _prep_core(inputs[e * C : (e + 1) * C], w1[e], b1[e], w2[e], b2[e])
        for e in range(E)
    ]


def kernel_run(inputs, w1, b1, w2, b2, trace=False, **trace_kwargs):
    """Run on 8 NeuronCores; returns (full_output [T, D], BassKernelResults)."""
    inputs = np.asarray(inputs, dtype=np.float32)
    w1 = np.asarray(w1, dtype=np.float32)
    b1 = np.asarray(b1, dtype=np.float32)
    w2 = np.asarray(w2, dtype=np.float32)
    b2 = np.asarray(b2, dtype=np.float32)
    nc = build_bass()
    res = run_bass_kernel_spmd(
        nc,
        _in_maps(inputs, w1, b1, w2, b2),
        core_ids=list(range(E)),
        trace=trace,
        **trace_kwargs,
    )
    # y is stored transposed [D_T, P, C] = yT[d, c]; undo per core.
    out = np.concatenate(
        [res.results[e]["y"].reshape(D, C).astype(np.float32).T for e in range(E)],
        axis=0,
    )
    return np.ascontiguousarray(out), res


def kernel(inputs, w1, b1, w2, b2):
    out, _ = kernel_run(inputs, w1, b1, w2, b2, trace=False)
    return out
